# revision 99
# baseline (speedup 1.0000x reference)
"""Trainium2 Bass kernel for nn_MultiHeadAttention_36223754174786.

Fused transformer block: QKV projection -> 16-head attention (naive, full
[S,S] scores) -> LayerNorm -> FeedForward (relu MLP) with residual.
B=2, S=2048, D=1024, H=16, DK=64, FF_HIDDEN=2048.

Sharding: data-parallel over tokens across 8 NeuronCores.  Core c handles 512
query tokens of batch b=c//4.  K/V projections for the full batch are
recomputed on each core (replicated inside the 4-core batch group).

Numerics / PE-cost tricks (cost model: matmul = out_free x cycles_per_row,
fp8e4+DoubleRow = 0.5 cyc/row with 256-deep contraction):
  * Q/K/V projections run as fp8e4 DoubleRow 3-term products:
    x ~ x_hi + x_lo, w ~ w_hi + w_lo (split on host, residual quantized
    without rescale -- lands in fp8 subnormals, total ~0.1% vs bf16's 0.4%),
    accumulating x_hi@w_hi + x_lo@w_hi + x_hi@w_lo in fp32 PSUM.  Weights are
    host-scaled x64 into fp8 range; the 1/64 descale folds into the existing
    bias pass for q/k, and for v into the softmax-denominator trick (the
    "ones" column of [V|1] is 64, so numerator and denominator carry the
    same x64 and it cancels).
  * attention is computed token-major: attn[q,65] accumulates
    expT[keys,q].T @ [V|1] with bf16 exp (ACT writes bf16 directly) and bf16
    V as the 65-wide moving operand -- half the PE rows of the [65,512]
    layout and no PE transposes or copies afterwards.
  * attention emission is software-pipelined: scores for head h+1 lead
    attnV(h) by 2 exp-units so ACT (exp, the 2nd-busiest engine) never
    starves, and the next pair's fp8 K-projection chunks are interleaved
    into the exp-paced stalls of the scores stream.
  * LayerNorm via bn_stats/bn_aggr; ln_g==1/ln_b==0 and zero biases are
    specialized away at build time (runtime-checked).  ffi and the final
    output are bf16 (bf16 PE transposes at 1 cyc/row, half-size out DMA).
  * FFN1 = relu(W1.T @ ffiT) in bf16 with W1 host-scaled 1/8 so hT = h/8
    lands in fp8 range; hT splits hi/lo (DVE relu -> ACT copy -> DVE
    residual) and FFN2 runs as 3-term fp8 DoubleRow against host-split
    8*W2 (product exactly ff, no descale), outputs DMA'd as they finish.

Measured: 283182 ns (TimelineSim), rel err 6.0e-3 on hardware.
Structure: V projection first (wv_hi and the tg0 xv tiles lead the DMA
queue, matching mm3 term order; Q/K operands and consts stream in via the
p3 stage hooks), then Q projection for pairs 0..2, then software-pipelined
attention whose stall-fill computes the K projection for pairs 1..7 AND
the Q projection for pairs 3..7, then LayerNorm -> FFN.

Cost-model-vs-hardware divergence map (sim passes, HW wrong -- bisected):
  * M_OUTER=True -- attnV with m outer interleaves the four per-q PSUM
    accumulation groups inside ONE PSUM tile (~8us faster in sim,
    rel err 0.23 on HW).  Interleaving groups across DIFFERENT banks is
    fine (the kproj/qproj fill does it everywhere).
  * Deferring Q-projection pairs m<=2 into the attention fill breaks on
    HW; m>=3 is safe (this build defers 3..7).  The error scales with how
    many early pairs are deferred (m=2 only: absmax err 0.98; m=1..2:
    1.79), and moving qproj(2) to the END of window 1 (same write->read
    slack as the passing m=3 case) instead produces NaN -- a third
    failure mode.  m<=2 deferral is unsafe under every tested schedule;
    the dependence on absolute emission position suggests a Tile
    sem-assignment edge in the first attention windows.  Root-cause
    before touching it.
  * The prefix is DMA-byte-bound; prefix schedule shuffles without the
    v-first reorder are sim-neutral.

Explored dead ends (do not retry): PSUM->DRAM DMA is rejected by bass
(source must be SBUF/DRAM), so the FFN2 tail's residual-in-PSUM +
direct-DMA trick is impossible; splitting the final adds loses to the
250ns PSUM-access init per DVE op; the CollectiveCompute cost model
(15us + 40GB/s) makes collective-based K/V sharing slower than local
recompute -- only raw remote_dma (180GB/s model) could beat it.
"""

import numpy as np

import concourse.bass as bass
import concourse.tile as tile
from concourse import bacc, mybir
from concourse.bass_utils import run_bass_kernel_spmd
from concourse.masks import make_identity

F32 = mybir.dt.float32
F32R = mybir.dt.float32r
BF16 = mybir.dt.bfloat16
FP8 = mybir.dt.float8e4
DR = mybir.MatmulPerfMode.DoubleRow
W8SCALE = 64.0
AF = mybir.ActivationFunctionType
OP = mybir.AluOpType

B, S, D, H = 2, 2048, 1024, 16
DK = D // H          # 64
FF = 2048
P = 128
T = 512              # query tokens per core
N_CORES = 8
KC = S // P          # 16 key chunks
QS = T // P          # 4 query sub-tiles
DCH = D // P         # 8 chunks of the model dim
NDR = DCH // 2       # DoubleRow steps over the model dim
FFC = FF // P        # 16 chunks of the ffn hidden dim
ALL_PHASES = ("p1", "p3", "pa", "ln", "tr", "ffn")
M_OUTER = False      # attnV loop order: m outer (frees exp tiles early)
BF16_TAIL = True    # bf16 ffi/transposes/output DMA


def _bcast_ap(ap):
    """Partition-broadcast a 1-D DRAM vector to [128, n] for DMA."""
    return bass.AP(tensor=ap.tensor, offset=ap.offset, ap=[[0, P]] + list(ap.ap))


def build_program(phases=ALL_PHASES, ln_affine=True, b2_zero=False):
    phases = set(phases)
    nc = bacc.Bacc("TRN2", target_bir_lowering=False, debug=False,
                   num_devices=N_CORES)

    def mm(out_ap, lhsT, rhs, start, stop, perf_mode=None):
        nc.tensor.matmul(out_ap, lhsT, rhs, start=start, stop=stop,
                         perf_mode=perf_mode)

    def mm3(ps, w2_, x2_, wsl, xsl):
        """3-term fp8 DoubleRow accumulation: ps = x@w over the full model
        dim.  w2_/x2_ are (hi, lo) tiles shaped [P, DCH, n]; wsl/xsl slice
        their last axis.  lhsT = w-side (stationary), rhs = x-side."""
        wh, wl = w2_
        xh, xl = x2_
        terms = ((wh, xh), (wl, xh), (wh, xl))
        for t, (wb, xb) in enumerate(terms):
            for j in range(NDR):
                mm(ps, wb[:, 2 * j:2 * j + 2, wsl], xb[:, 2 * j:2 * j + 2, xsl],
                   start=(t == 0 and j == 0), stop=(t == 2 and j == NDR - 1),
                   perf_mode=DR)

    xq2 = [nc.dram_tensor(n, [D, T], FP8, kind="ExternalInput")
           for n in ("xqh", "xql")]
    xk2 = [nc.dram_tensor(n, [D, S], FP8, kind="ExternalInput")
           for n in ("xkh", "xkl")]
    xv2 = [nc.dram_tensor(n, [D, S], FP8, kind="ExternalInput")
           for n in ("xvh", "xvl")]
    wq2 = [nc.dram_tensor(n, [D, D], FP8, kind="ExternalInput")
           for n in ("wqh", "wql")]
    wk2 = [nc.dram_tensor(n, [D, D], FP8, kind="ExternalInput")
           for n in ("wkh", "wkl")]
    wv2 = [nc.dram_tensor(n, [D, D], FP8, kind="ExternalInput")
           for n in ("wvh", "wvl")]
    w1 = nc.dram_tensor("w1", [D, FF], BF16, kind="ExternalInput")
    w22 = [nc.dram_tensor(n, [FF, D], FP8, kind="ExternalInput")
           for n in ("w2h", "w2l")]
    bq = nc.dram_tensor("bq", [D], F32, kind="ExternalInput")   # x64 on host
    bk = nc.dram_tensor("bk", [D], F32, kind="ExternalInput")   # x64 on host
    bv = nc.dram_tensor("bv", [D], F32, kind="ExternalInput")   # x64 on host
    b1 = nc.dram_tensor("b1", [FF], F32, kind="ExternalInput")
    b2 = nc.dram_tensor("b2", [D], F32, kind="ExternalInput")
    ln_g = nc.dram_tensor("ln_g", [D], F32, kind="ExternalInput")
    ln_b = nc.dram_tensor("ln_b", [D], F32, kind="ExternalInput")
    out = nc.dram_tensor("out", [T, D], BF16 if BF16_TAIL else F32,
                         kind="ExternalOutput")

    def chunked(dram):
        """DRAM [C*P, n] viewed as [P, C, n] for a single chunked DMA."""
        return dram[:].rearrange("(c p) n -> p c n", p=P)

    def emit_p1(qT, bq_col, xq_t, wq_t, acc):
        """Q projection for head-pairs 0..2; m=3..7 are deferred into the
        attention fill (each qT[m] is only needed when pair m's scores
        start).  Runs AFTER p3 on operands prefetched by the p3 hooks."""
        for m in range(3):
            ps = acc.tile([P, 512], F32, tag="acc", name="acc")
            mm3(ps, wq_t, xq_t, slice(m * P, (m + 1) * P), slice(None))
            # qT = ps/64 + bq  (bq arrives x64 from host)
            nc.vector.tensor_scalar(qT[m], ps, bq_col[:, m:m + 1],
                                    1.0 / W8SCALE, OP.add, OP.mult)

    def emit_p3(v_sb, bv_b, ones_t, p3w, p3x, acc, prefetch=None):
        wv_t = [p3w.tile([P, DCH, D], FP8, tag=f"wv{i}", name=f"wv{i}")
                for i in range(2)]
        nc.sync.dma_start(wv_t[0], chunked(wv2[0]))
        for tg in range(KC // 4):
            xv_t = []
            for i in range(2):
                x_ = p3x.tile([P, DCH, 512], FP8, tag=f"xv{i}", name=f"xv{i}")
                nc.sync.dma_start(
                    x_, xv2[i][:, tg * 512:(tg + 1) * 512].rearrange(
                        "(c p) n -> p c n", p=P))
                xv_t.append(x_)
            if tg == 0:
                nc.sync.dma_start(wv_t[1], chunked(wv2[1]))
            if prefetch is not None and tg in prefetch:
                prefetch[tg]()
            for ti in range(4):
                t = tg * 4 + ti
                nc.vector.tensor_copy(v_sb[t][:, :, DK:DK + 1], ones_t)
                for dch in range(2):
                    ps = acc.tile([P, 512], F32, tag="acc", name="acc")
                    # lhsT = xv tokens (stationary), rhs = wv cols (moving)
                    mm3(ps, xv_t, wv_t, slice(ti * P, (ti + 1) * P),
                        slice(dch * 512, (dch + 1) * 512))
                    # v' = 64*v: ps is x64 (weights) and bv arrives x64; the
                    # softmax ones-column is 64 so the scale cancels.
                    nc.vector.tensor_tensor(
                        v_sb[t][:, dch * 8:(dch + 1) * 8, 0:DK],
                        ps[:].rearrange("p (h d) -> p h d", h=8),
                        bv_b[:, dch * 512:(dch + 1) * 512].rearrange(
                            "p (h d) -> p h d", h=8),
                        OP.add)

    def load_qk(p1w, p2w, load_consts):
        """Staged Q/K loads keyed by p3 token-group, riding behind p3's own
        xv/wv traffic.  Order matches first use: stage0 feeds the post-p3
        Q-projection's hi terms (and the consts), stage1 the attention
        prologue's (wk_hi, xk_hi), then the lo parts."""
        xq_t = [p1w.tile([P, DCH, T], FP8, tag=f"xq{i}", name=f"xq{i}")
                for i in range(2)]
        wq_t = [p1w.tile([P, DCH, D], FP8, tag=f"wq{i}", name=f"wq{i}")
                for i in range(2)]
        xk_t = [p2w.tile([P, DCH, S], FP8, tag=f"xk{i}", name=f"xk{i}")
                for i in range(2)]
        wk_t = [p2w.tile([P, DCH, D], FP8, tag=f"wk{i}", name=f"wk{i}")
                for i in range(2)]
        stages = {
            0: lambda: (load_consts(),
                        nc.sync.dma_start(xq_t[0], chunked(xq2[0])),
                        nc.sync.dma_start(wq_t[0], chunked(wq2[0]))),
            1: lambda: (nc.sync.dma_start(xk_t[0], chunked(xk2[0])),
                        nc.sync.dma_start(wk_t[0], chunked(wk2[0]))),
            2: lambda: (nc.sync.dma_start(wq_t[1], chunked(wq2[1])),
                        nc.sync.dma_start(wk_t[1], chunked(wk2[1]))),
            3: lambda: (nc.sync.dma_start(xq_t[1], chunked(xq2[1])),
                        nc.sync.dma_start(xk_t[1], chunked(xk2[1]))),
        }
        return xq_t, wq_t, xk_t, wk_t, stages

    def emit_p2_attn(qT, v_sb, attn, bq_col, bk_col, xq_t, wq_t, xk_t, wk_t,
                     aK, aE, aR, acc, psS, psA):
        """Software-pipelined attention: 128 exp-units (head, score-group),
        ACT-paced via psS double-buffering; attnV(h) trails its last exp by
        2 units; the next pair's fp8 K- AND Q-projection chunks fill the
"""
        kp_tiles = {}
        exps = {}
        acc_open = {}
        TERMS = ((wk_t[0], xk_t[0]), (wk_t[1], xk_t[0]), (wk_t[0], xk_t[1]))
        QTERMS = ((wq_t[0], xq_t[0]), (wq_t[1], xq_t[0]), (wq_t[0], xq_t[1]))

        def emit_qproj_sub(m, term):
            """One term of deferred Q-projection for head-pair m."""
            if term == 0:
                acc_open["q", m] = acc.tile([P, 512], F32, tag="acc",
                                            name="acc")
            ps = acc_open["q", m]
            wb, xb = QTERMS[term]
            for j in range(NDR):
                mm(ps, wb[:, 2 * j:2 * j + 2, m * P:(m + 1) * P],
                   xb[:, 2 * j:2 * j + 2, :],
                   start=(term == 0 and j == 0),
                   stop=(term == 2 and j == NDR - 1), perf_mode=DR)
            if term == 2:
                nc.vector.tensor_scalar(qT[m], ps, bq_col[:, m:m + 1],
                                        1.0 / W8SCALE, OP.add, OP.mult)

        def emit_kproj_sub(p, nch, term):
            """One term (4 DR matmuls, ~0.43us PE) of K-projection chunk
            (p, nch) -- fine-grained so it can pack into the exp-paced
            stalls of the scores stream."""
            if nch == 0 and term == 0:
                kp_tiles[p] = aK.tile([P, S], F32R, tag="kp", name="kp")
            if term == 0:
                acc_open[p] = acc.tile([P, 512], F32, tag="acc", name="acc")
            ps = acc_open[p]
            wb, xb = TERMS[term]
            for j in range(NDR):
                mm(ps, wb[:, 2 * j:2 * j + 2, p * P:(p + 1) * P],
                   xb[:, 2 * j:2 * j + 2, nch * 512:(nch + 1) * 512],
                   start=(term == 0 and j == 0),
                   stop=(term == 2 and j == NDR - 1), perf_mode=DR)
            if term == 2:
                # kp = ps/64 + bk  (bk arrives x64)
                nc.vector.tensor_scalar(
                    kp_tiles[p][:, nch * 512:(nch + 1) * 512], ps,
                    bk_col[:, p:p + 1], 1.0 / W8SCALE, OP.add, OP.mult)

        def emit_unit(u):
            head, g = u // 8, u % 8
            p, hp = head // 2, head % 2
            lo, hi = hp * DK, (hp + 1) * DK
            kp = kp_tiles[p]
            ps = psS.tile([P, 1024], F32, tag="psS", name="psS")
            for j in range(2):
                m = 2 * g + j
                mm(ps[:, j * 512:(j + 1) * 512],
                   kp[lo:hi, m * P:(m + 1) * P],
                   qT[p][lo:hi, :], start=True, stop=True)
            e = aE.tile([P, 1024], BF16, tag="exp", name="exp")
            nc.scalar.activation(e, ps, AF.Exp)
            exps.setdefault(head, []).append(e)

        def emit_attnv(head):
            ex = exps.pop(head)
            pa = psA.tile([P, QS, DK + 1], F32, tag="pa", name="pa")
            # m outer so early exp tiles are released after 8 matmuls and
            # ACT can start on the next head's scores immediately
            loop = ([(m, q) for m in range(KC) for q in range(QS)]
                    if M_OUTER else
                    [(m, q) for q in range(QS) for m in range(KC)])
            for m, q in loop:
                mm(pa[:, q, :],
                   ex[m // 2][:, (m % 2) * 512 + q * P:
                              (m % 2) * 512 + (q + 1) * P],
                   v_sb[m][:, head, :],
                   start=(m == 0), stop=(m == KC - 1))
            rc = aR.tile([P, QS], F32, tag="rc", name="rc")
            nc.vector.reciprocal(rc, pa[:, :, DK:DK + 1])
            for q in range(QS):
                nc.vector.tensor_scalar_mul(
                    attn[q][:, head * DK:(head + 1) * DK],
                    pa[:, q, 0:DK], rc[:, q:q + 1])

        for nch in range(4):
            for term in range(3):
                emit_kproj_sub(0, nch, term)
        subs = []
        for w in range(1, H // 2):
            if w >= 3:
                subs += [("q", w, term) for term in range(3)]
            subs += [("k", w, nch, term) for nch in range(4)
                     for term in range(3)]

        def emit_sub(s):
            if s[0] == "q":
                emit_qproj_sub(s[1], s[2])
            else:
                emit_kproj_sub(s[1], s[2], s[3])

        sc = 0
        for idx in range(130):
            if idx < 128:
                emit_unit(idx)
            if idx % 8 != 1 and sc < len(subs) \
                    and subs[sc][1] <= idx // 16 + 1:
                emit_sub(subs[sc])
                sc += 1
            if idx >= 9 and (idx - 9) % 8 == 0:
                emit_attnv((idx - 9) // 8)
        while sc < len(subs):
            emit_sub(subs[sc])
            sc += 1
    def emit_ln_tr(attn, ffi, ffiT, eps_t, lng_b, lnb_b, ident_bf, lnp, psTr):
        for q in range(QS):
            stats = lnp.tile([P, 2, 6], F32, tag="stats", name="stats")
            for sg in range(2):
                nc.vector.bn_stats(stats[:, sg, :],
                                   attn[q][:, sg * 512:(sg + 1) * 512])
            mv = lnp.tile([P, 2], F32, tag="mv", name="mv")
            nc.vector.bn_aggr(mv, stats)
            std = lnp.tile([P, 1], F32, tag="std", name="std")
            nc.scalar.activation(std, mv[:, 1:2], AF.Sqrt, bias=eps_t)
            rstd = lnp.tile([P, 1], F32, tag="rstd", name="rstd")
            nc.vector.reciprocal(rstd, std)
            nc.vector.tensor_scalar(ffi[q], attn[q], mv[:, 0:1], rstd,
                                    OP.subtract, OP.mult)
            if ln_affine:
                nc.vector.tensor_mul(ffi[q], ffi[q], lng_b)
                nc.vector.tensor_add(ffi[q], ffi[q], lnb_b)
            for k in range(DCH):
                pt = psTr.tile([P, P], BF16 if BF16_TAIL else F32,
                               tag="ptr", name="ptr")
                nc.tensor.transpose(pt, ffi[q][:, k * P:(k + 1) * P],
                                    ident_bf[:, 0:P])
                nc.vector.tensor_copy(ffiT[k][:, q * P:(q + 1) * P], pt)

    def emit_ffn(ffi, ffiT, out_sb, b1_col, b2_b,
                 hp_, fw1, fw2, psH, psF, out_dma=None):
        """FFN1 in bf16 (W1 host-scaled 1/8 so hT = h/8 sits in fp8 range);
        hT is split hi/lo into fp8 (DVE relu -> ACT copy -> DVE residual) and
        FFN2 runs as 3-term fp8 DoubleRow against host-split 8*W2, so the
        product is exactly ff with no descale pass."""
        hT8 = [hp_.tile([P, FFC, T], FP8, tag=f"hT8{i}", name=f"hT8{i}")
               for i in range(2)]
        w1_sb = []
        for k in range(DCH):
            w1t = fw1.tile([P, FF], BF16, tag=f"w1_{k}", name=f"w1_{k}")
            nc.sync.dma_start(w1t, w1[k * P:(k + 1) * P, :])
            w1_sb.append(w1t)

        def w2_tile(i, g, half):
            t_ = fw2.tile([P, 2, 512], FP8, tag=f"w2{half}{i}",
                          name=f"w2{half}{i}")
            nc.sync.dma_start(
                t_, w22[i][2 * g * P:(2 * g + 2) * P,
                           half * 512:(half + 1) * 512].rearrange(
                    "(c p) n -> p c n", p=P))
            return t_

        def ffn2_pair(pss, g, w2h_t, w2l_t, on_q_done=None):
            terms = ((0, w2h_t), (1, w2h_t), (0, w2l_t))
            for q in range(QS):
                for tr, (hx, wx) in enumerate(terms):
                    mm(pss[q], hT8[hx][:, 2 * g:2 * g + 2, q * P:(q + 1) * P],
                       wx, start=(g == 0 and tr == 0),
                       stop=(g == FFC // 2 - 1 and tr == 2), perf_mode=DR)
                if on_q_done is not None:
                    on_q_done(q)

        pss0 = [psF.tile([P, 512], F32, tag="psF", name="psF")
                for _ in range(QS)]
        for fk in range(FFC):
            ps = psH.tile([P, T], F32, tag="psH", name="psH")
            for k in range(DCH):
                mm(ps, w1_sb[k][:, fk * P:(fk + 1) * P], ffiT[k],
                   start=(k == 0), stop=(k == DCH - 1))
            t_re = fw2.tile([P, T], F32, tag="tre", name="tre")
            nc.vector.tensor_scalar(t_re, ps, b1_col[:, fk:fk + 1], 0.0,
                                    OP.add, OP.max)
            nc.scalar.activation(hT8[0][:, fk, :], t_re, AF.Copy)
            nc.vector.tensor_tensor(hT8[1][:, fk, :], t_re,
                                    hT8[0][:, fk, :], OP.subtract)
            if fk % 2 == 1:
                g = fk // 2
                ffn2_pair(pss0, g, w2_tile(0, g, 0), w2_tile(1, g, 0))
        for q in range(QS):
            nc.vector.tensor_add(out_sb[q][:, 0:512], pss0[q],
                                 ffi[q][:, 0:512])
            if not b2_zero:
                nc.vector.tensor_add(out_sb[q][:, 0:512],
                                     out_sb[q][:, 0:512], b2_b[:, 0:512])
            if out_dma is not None:
                out_dma(q, 0)
        pss1 = [psF.tile([P, 512], F32, tag="psF", name="psF")
                for _ in range(QS)]

        def final_add(q):
            nc.vector.tensor_add(out_sb[q][:, 512:1024], pss1[q],
                                 ffi[q][:, 512:1024])
            if not b2_zero:
                nc.vector.tensor_add(out_sb[q][:, 512:1024],
                                     out_sb[q][:, 512:1024],
                                     b2_b[:, 512:1024])
            if out_dma is not None:
                out_dma(q, 1)

        for g in range(FFC // 2):
            ffn2_pair(pss1, g, w2_tile(0, g, 1), w2_tile(1, g, 1),
                      on_q_done=final_add if g == FFC // 2 - 1 else None)

    with tile.TileContext(nc) as tc:
        with (
            tc.tile_pool(name="const", bufs=1) as cp,
            tc.tile_pool(name="qTp", bufs=1) as qp,
            tc.tile_pool(name="attnp", bufs=1) as ap_,
            tc.tile_pool(name="p1w", bufs=1) as p1w,
            tc.tile_pool(name="accp", bufs=2, space="PSUM") as acc,
        ):
            ident = cp.tile([P, P], F32, tag="ident", name="ident")
            make_identity(nc, ident)
            ident_bf = cp.tile([P, P], BF16, tag="identb", name="identb")
            nc.vector.tensor_copy(ident_bf, ident)
            eps_t = cp.tile([P, 1], F32, tag="eps", name="eps")
            nc.vector.memset(eps_t, 1e-5)
            ones_t = cp.tile([P, H, 1], F32, tag="ones", name="ones")
            nc.vector.memset(ones_t, W8SCALE)
            bq_col = cp.tile([P, DCH], F32, tag="bqc", name="bqc")
            bk_col = cp.tile([P, DCH], F32, tag="bkc", name="bkc")
            b1_col = cp.tile([P, FFC], F32, tag="b1c", name="b1c")
            lng_b = cp.tile([P, D], F32, tag="lng", name="lng")
            lnb_b = cp.tile([P, D], F32, tag="lnb", name="lnb")
            bv_b = cp.tile([P, D], F32, tag="bvb", name="bvb")
            b2_b = cp.tile([P, D], F32, tag="b2b", name="b2b")

            def load_consts():
                # issued AFTER p1's operand DMAs: the bcasts are 4x512KB and
                # would otherwise interleave on the DMA engines ahead of
                # wq_hi, delaying the first matmul.  bv leads the gpsimd
                # queue (cumulative sem; its consumer is p3's first bias).
                nc.sync.dma_start(bq_col, bq[:].rearrange("(o p) -> p o", p=P))
                nc.sync.dma_start(bk_col, bk[:].rearrange("(o p) -> p o", p=P))
                nc.sync.dma_start(b1_col, b1[:].rearrange("(o p) -> p o", p=P))
                nc.gpsimd.dma_start(bv_b, _bcast_ap(bv[:]))
                nc.gpsimd.dma_start(lng_b, _bcast_ap(ln_g[:]))
                nc.gpsimd.dma_start(lnb_b, _bcast_ap(ln_b[:]))
                nc.gpsimd.dma_start(b2_b, _bcast_ap(b2[:]))

            qT = [qp.tile([P, T], F32R, tag=f"qT{m}", name=f"qT{m}")
                  for m in range(DCH)]
            attn = [ap_.tile([P, D], F32, tag=f"attn{q}", name=f"attn{q}")
                    for q in range(QS)]

            with tc.tile_pool(name="vp", bufs=1) as vp:
                v_sb = [vp.tile([P, H, DK + 1], BF16, tag=f"v{t}", name=f"v{t}")
                        for t in range(KC)]
                with tc.tile_pool(name="p2w", bufs=1) as p2w:
                    if "pa" in phases or "p1" in phases:
                        xq_t, wq_t, xk_t, wk_t, stages = load_qk(
                            p1w, p2w, load_consts)
                    else:
                        load_consts()
                    if "p3" in phases:
                        with (
                            tc.tile_pool(name="p3w", bufs=1) as p3w,
                            tc.tile_pool(name="p3x", bufs=3) as p3x,
                        ):
                            emit_p3(v_sb, bv_b, ones_t, p3w, p3x, acc,
                                    prefetch=stages)
                    else:
                        for tg in (0, 1, 2, 3):
                            stages[tg]()
                    if "p1" in phases:
                        emit_p1(qT, bq_col, xq_t, wq_t, acc)
                    if "pa" in phases:
                        with (
                            tc.tile_pool(name="aK", bufs=3) as aK,
                            tc.tile_pool(name="aE", bufs=15) as aE,
                            tc.tile_pool(name="aR", bufs=2) as aR,
                            tc.tile_pool(name="psS", bufs=2, space="PSUM") as psS,
                            tc.tile_pool(name="psA", bufs=2, space="PSUM") as psA,
                        ):
                            emit_p2_attn(qT, v_sb, attn, bq_col, bk_col,
                                         xq_t, wq_t, xk_t, wk_t,
                                         aK, aE, aR, acc, psS, psA)
                            # prewarm the Sqrt ACT table set so the switch
                            # isn't on the LayerNorm critical path
                            warm = aR.tile([P, 1], F32, tag="warm",
                                           name="warm")
                            nc.scalar.activation(warm, eps_t, AF.Sqrt)

            with (
                tc.tile_pool(name="ffip", bufs=1) as fip,
                tc.tile_pool(name="ffiTp", bufs=1) as ftp,
                tc.tile_pool(name="outp", bufs=1) as op_,
            ):
                ffi = [fip.tile([P, D], BF16 if BF16_TAIL else F32,
                                tag=f"ffi{q}", name=f"ffi{q}")
                       for q in range(QS)]
                ffiT = [ftp.tile([P, T], BF16, tag=f"ffiT{k}", name=f"ffiT{k}")
                        for k in range(DCH)]
                out_sb = [op_.tile([P, D], BF16 if BF16_TAIL else F32,
                                   tag=f"out{q}", name=f"out{q}")
                          for q in range(QS)]

                if "ln" in phases and "tr" in phases:
                    with (
                        tc.tile_pool(name="lnp", bufs=4) as lnp,
                        tc.tile_pool(name="psTr", bufs=4, space="PSUM") as psTr,
                    ):
                        emit_ln_tr(attn, ffi, ffiT, eps_t, lng_b, lnb_b,
                                   ident_bf if BF16_TAIL else ident,
                                   lnp, psTr)

                if "ffn" in phases:
                    with (
                        tc.tile_pool(name="hTp", bufs=1) as hp_,
                        tc.tile_pool(name="fw1", bufs=1) as fw1,
                        tc.tile_pool(name="fw2", bufs=4) as fw2,
                        tc.tile_pool(name="psH", bufs=2, space="PSUM") as psH,
                        tc.tile_pool(name="psF", bufs=4, space="PSUM") as psF,
                    ):
                        def out_dma(q, half):
                            sl = slice(half * 512, (half + 1) * 512)
                            nc.sync.dma_start(out[q * P:(q + 1) * P, sl],
                                              out_sb[q][:, sl])
                        emit_ffn(ffi, ffiT, out_sb, b1_col, b2_b,
                                 hp_, fw1, fw2, psH, psF, out_dma=out_dma)

    nc.compile()
    return nc


def _split8(a):
    """f32 array -> (hi, lo) float8_e4m3 pair with hi + lo ~= a."""
    import ml_dtypes
    hi = a.astype(ml_dtypes.float8_e4m3)
    lo = (a - hi.astype(np.float32)).astype(ml_dtypes.float8_e4m3)
    return hi, lo


def kernel(**inputs) -> np.ndarray:
    import ml_dtypes
    f32 = lambda a: np.asarray(a, dtype=np.float32)
    query, key, value = f32(inputs["query"]), f32(inputs["key"]), f32(inputs["value"])
    scale = 1.0 / np.sqrt(np.float32(DK))
    wqh, wql = _split8(np.ascontiguousarray(f32(inputs["Wq"]) * (scale * W8SCALE)))
    wkh, wkl = _split8(f32(inputs["Wk"]) * W8SCALE)
    wvh, wvl = _split8(f32(inputs["Wv"]) * W8SCALE)
    bq = f32(inputs["bq"]) * scale * W8SCALE
    bk = f32(inputs["bk"]) * W8SCALE
    bv = f32(inputs["bv"]) * W8SCALE
    # W1 scaled 1/8 so hT = h/8 lands in fp8 range; W2 x8 compensates exactly
    w1 = (f32(inputs["W1"]) * 0.125).astype(ml_dtypes.bfloat16)
    b1 = f32(inputs["b1"]) * 0.125
    w2h, w2l = _split8(f32(inputs["W2"]) * 8.0)
    b2 = f32(inputs["b2"])
    ln_g, ln_b = f32(inputs["ln_g"]), f32(inputs["ln_b"])

    ln_affine = not (np.all(ln_g == 1.0) and np.all(ln_b == 0.0))
    nc = build_program(ln_affine=ln_affine, b2_zero=not b2.any())

    shared = dict(wqh=wqh, wql=wql, wkh=wkh, wkl=wkl, wvh=wvh, wvl=wvl,
                  w1=w1, w2h=w2h, w2l=w2l, bq=bq, bk=bk, bv=bv,
                  b1=b1, b2=b2, ln_g=ln_g, ln_b=ln_b)
    xk_splits, xv_splits = {}, {}
    for b in range(B):
        xk_splits[b] = _split8(np.ascontiguousarray(key[b].T))
        xv_splits[b] = _split8(np.ascontiguousarray(value[b].T))
    in_maps = []
    for c in range(N_CORES):
        b = c // 4
        t0 = (c % 4) * T
        xqh, xql = _split8(np.ascontiguousarray(query[b, t0:t0 + T, :].T))
        in_maps.append(dict(
            xqh=xqh, xql=xql,
            xkh=xk_splits[b][0], xkl=xk_splits[b][1],
            xvh=xv_splits[b][0], xvl=xv_splits[b][1],
            **shared,
        ))

    res = run_bass_kernel_spmd(nc, in_maps, list(range(N_CORES)))
    out = np.empty((B, S, D), dtype=np.float32)
    for c in range(N_CORES):
        b = c // 4
        t0 = (c % 4) * T
        out[b, t0:t0 + T, :] = res.results[c]["out"].astype(np.float32)
    return out


# revision 103
# speedup vs baseline: 1.0270x; 1.0270x over previous
"""Trainium2 Bass kernel for nn_MultiHeadAttention_36223754174786.

Fused transformer block: QKV projection -> 16-head attention (naive, full
[S,S] scores) -> LayerNorm -> FeedForward (relu MLP) with residual.
B=2, S=2048, D=1024, H=16, DK=64, FF_HIDDEN=2048.

Sharding: data-parallel over tokens across 8 NeuronCores.  Core c handles 512
query tokens of batch b=c//4.  K/V projections for the full batch are
recomputed on each core (replicated inside the 4-core batch group).

Numerics / PE-cost tricks (cost model: matmul = out_free x cycles_per_row,
fp8e4+DoubleRow = 0.5 cyc/row with 256-deep contraction):
  * Q/K/V projections run as fp8e4 DoubleRow 3-term products:
    x ~ x_hi + x_lo, w ~ w_hi + w_lo (split on host, residual quantized
    without rescale -- lands in fp8 subnormals, total ~0.1% vs bf16's 0.4%),
    accumulating x_hi@w_hi + x_lo@w_hi + x_hi@w_lo in fp32 PSUM.  Weights are
    host-scaled x64 into fp8 range; the 1/64 descale folds into the existing
    bias pass for q/k, and for v into the softmax-denominator trick (the
    "ones" column of [V|1] is 64, so numerator and denominator carry the
    same x64 and it cancels).
  * attention is computed token-major: attn[q,65] accumulates
    expT[keys,q].T @ [V|1] with bf16 exp (ACT writes bf16 directly) and bf16
    V as the 65-wide moving operand -- half the PE rows of the [65,512]
    layout and no PE transposes or copies afterwards.
  * attention emission is software-pipelined: scores for head h+1 lead
    attnV(h) by 2 exp-units so ACT (exp, the 2nd-busiest engine) never
    starves, and the next pair's fp8 K-projection chunks are interleaved
    into the exp-paced stalls of the scores stream.
  * LayerNorm via bn_stats/bn_aggr; ln_g==1/ln_b==0 and zero biases are
    specialized away at build time (runtime-checked).  ffi and the final
    output are bf16 (bf16 PE transposes at 1 cyc/row, half-size out DMA).
  * FFN1 = relu(W1.T @ ffiT) in bf16 with W1 host-scaled 1/8 so hT = h/8
    lands in fp8 range; hT splits hi/lo (DVE relu -> ACT copy -> DVE
    residual) and FFN2 runs as 3-term fp8 DoubleRow against host-split
    8*W2 (product exactly ff, no descale), outputs DMA'd as they finish.

Measured: 283182 ns (TimelineSim), rel err 6.0e-3 on hardware.
Structure: V projection first (wv_hi and the tg0 xv tiles lead the DMA
queue, matching mm3 term order; Q/K operands and consts stream in via the
p3 stage hooks), then Q projection for pairs 0..2, then software-pipelined
attention whose stall-fill computes the K projection for pairs 1..7 AND
the Q projection for pairs 3..7, then LayerNorm -> FFN.

Cost-model-vs-hardware divergence map (sim passes, HW wrong -- bisected):
  * M_OUTER=True -- attnV with m outer interleaves the four per-q PSUM
    accumulation groups inside ONE PSUM tile (~8us faster in sim,
    rel err 0.23 on HW).  Interleaving groups across DIFFERENT banks is
    fine (the kproj/qproj fill does it everywhere).
  * Deferring Q-projection pairs m<=2 into the attention fill breaks on
    HW; m>=3 is safe (this build defers 3..7).  The error scales with how
    many early pairs are deferred (m=2 only: absmax err 0.98; m=1..2:
    1.79), and moving qproj(2) to the END of window 1 (same write->read
    slack as the passing m=3 case) instead produces NaN -- a third
    failure mode.  m<=2 deferral is unsafe under every tested schedule;
    the dependence on absolute emission position suggests a Tile
    sem-assignment edge in the first attention windows.  Root-cause
    before touching it.
  * The prefix is DMA-byte-bound; prefix schedule shuffles without the
    v-first reorder are sim-neutral.  The ~8.6us start ramp is a hard
    floor: Tile coalesces waits per accumulation GROUP (all operands of
    all its matmuls, regardless of emission order), so an early-term
    split of the first V-group cannot start before wv_lo/xv_lo arrive --
    tried, regressed, reverted.

Explored dead ends (do not retry): PSUM->DRAM DMA is rejected by bass
(source must be SBUF/DRAM), so the FFN2 tail's residual-in-PSUM +
direct-DMA trick is impossible; splitting the final adds loses to the
250ns PSUM-access init per DVE op; the CollectiveCompute cost model
(15us + 40GB/s) makes collective-based K/V sharing slower than local
recompute -- only raw remote_dma (180GB/s model) could beat it.
"""

import numpy as np

import concourse.bass as bass
import concourse.tile as tile
from concourse import bacc, mybir
from concourse.bass_utils import run_bass_kernel_spmd
from concourse.masks import make_identity

F32 = mybir.dt.float32
F32R = mybir.dt.float32r
BF16 = mybir.dt.bfloat16
FP8 = mybir.dt.float8e4
DR = mybir.MatmulPerfMode.DoubleRow
W8SCALE = 64.0
AF = mybir.ActivationFunctionType
OP = mybir.AluOpType

B, S, D, H = 2, 2048, 1024, 16
DK = D // H          # 64
FF = 2048
P = 128
T = 512              # query tokens per core
N_CORES = 8
KC = S // P          # 16 key chunks
QS = T // P          # 4 query sub-tiles
DCH = D // P         # 8 chunks of the model dim
NDR = DCH // 2       # DoubleRow steps over the model dim
FFC = FF // P        # 16 chunks of the ffn hidden dim
ALL_PHASES = ("p1", "p3", "pa", "ln", "tr", "ffn")
M_OUTER = False      # attnV loop order: m outer (frees exp tiles early)
BF16_TAIL = True    # bf16 ffi/transposes/output DMA


def _bcast_ap(ap):
    """Partition-broadcast a 1-D DRAM vector to [128, n] for DMA."""
    return bass.AP(tensor=ap.tensor, offset=ap.offset, ap=[[0, P]] + list(ap.ap))


def build_program(phases=ALL_PHASES, ln_affine=True, b2_zero=False):
    phases = set(phases)
    nc = bacc.Bacc("TRN2", target_bir_lowering=False, debug=False,
                   num_devices=N_CORES)

    def mm(out_ap, lhsT, rhs, start, stop, perf_mode=None):
        nc.tensor.matmul(out_ap, lhsT, rhs, start=start, stop=stop,
                         perf_mode=perf_mode)

    def mm3(ps, w2_, x2_, wsl, xsl):
        """3-term fp8 DoubleRow accumulation: ps = x@w over the full model
        dim.  w2_/x2_ are (hi, lo) tiles shaped [P, DCH, n]; wsl/xsl slice
        their last axis.  lhsT = w-side (stationary), rhs = x-side."""
        wh, wl = w2_
        xh, xl = x2_
        terms = ((wh, xh), (wl, xh), (wh, xl))
        for t, (wb, xb) in enumerate(terms):
            for j in range(NDR):
                mm(ps, wb[:, 2 * j:2 * j + 2, wsl], xb[:, 2 * j:2 * j + 2, xsl],
                   start=(t == 0 and j == 0), stop=(t == 2 and j == NDR - 1),
                   perf_mode=DR)

    xq2 = [nc.dram_tensor(n, [D, T], FP8, kind="ExternalInput")
           for n in ("xqh", "xql")]
    xk2 = [nc.dram_tensor(n, [D, S], FP8, kind="ExternalInput")
           for n in ("xkh", "xkl")]
    xv2 = [nc.dram_tensor(n, [D, S], FP8, kind="ExternalInput")
           for n in ("xvh", "xvl")]
    wq2 = [nc.dram_tensor(n, [D, D], FP8, kind="ExternalInput")
           for n in ("wqh", "wql")]
    wk2 = [nc.dram_tensor(n, [D, D], FP8, kind="ExternalInput")
           for n in ("wkh", "wkl")]
    wv2 = [nc.dram_tensor(n, [D, D], FP8, kind="ExternalInput")
           for n in ("wvh", "wvl")]
    w1 = nc.dram_tensor("w1", [D, FF], BF16, kind="ExternalInput")
    w22 = [nc.dram_tensor(n, [FF, D], FP8, kind="ExternalInput")
           for n in ("w2h", "w2l")]
    bq = nc.dram_tensor("bq", [D], F32, kind="ExternalInput")   # x64 on host
    bk = nc.dram_tensor("bk", [D], F32, kind="ExternalInput")   # x64 on host
    bv = nc.dram_tensor("bv", [D], F32, kind="ExternalInput")   # x64 on host
    b1 = nc.dram_tensor("b1", [FF], F32, kind="ExternalInput")
    b2 = nc.dram_tensor("b2", [D], F32, kind="ExternalInput")
    ln_g = nc.dram_tensor("ln_g", [D], F32, kind="ExternalInput")
    ln_b = nc.dram_tensor("ln_b", [D], F32, kind="ExternalInput")
    out = nc.dram_tensor("out", [T, D], BF16 if BF16_TAIL else F32,
                         kind="ExternalOutput")

    def chunked(dram):
        """DRAM [C*P, n] viewed as [P, C, n] for a single chunked DMA."""
        return dram[:].rearrange("(c p) n -> p c n", p=P)

    def emit_p1(qT, bq_col, xq_t, wq_t, acc):
        """Q projection for head-pairs 0..2; m=3..7 are deferred into the
        attention fill (each qT[m] is only needed when pair m's scores
        start).  Runs AFTER p3 on operands prefetched by the p3 hooks."""
        for m in range(3):
            ps = acc.tile([P, 512], F32, tag="acc", name="acc")
            mm3(ps, wq_t, xq_t, slice(m * P, (m + 1) * P), slice(None))
            # qT = ps/64 + bq  (bq arrives x64 from host)
            nc.vector.tensor_scalar(qT[m], ps, bq_col[:, m:m + 1],
                                    1.0 / W8SCALE, OP.add, OP.mult)

    def emit_p3(v_sb, bv_b, ones_t, p3w, p3x, acc, prefetch=None):
        wv_t = [p3w.tile([P, DCH, D], FP8, tag=f"wv{i}", name=f"wv{i}")
                for i in range(2)]
        nc.sync.dma_start(wv_t[0], chunked(wv2[0]))
        for tg in range(KC // 4):
            xv_t = []
            for i in range(2):
                x_ = p3x.tile([P, DCH, 512], FP8, tag=f"xv{i}", name=f"xv{i}")
                nc.sync.dma_start(
                    x_, xv2[i][:, tg * 512:(tg + 1) * 512].rearrange(
                        "(c p) n -> p c n", p=P))
                xv_t.append(x_)
            if tg == 0:
                nc.sync.dma_start(wv_t[1], chunked(wv2[1]))
            if prefetch is not None and tg in prefetch:
                prefetch[tg]()
            for ti in range(4):
                t = tg * 4 + ti
                nc.vector.tensor_copy(v_sb[t][:, :, DK:DK + 1], ones_t)
                for dch in range(2):
                    ps = acc.tile([P, 512], F32, tag="acc", name="acc")
                    # lhsT = xv tokens (stationary), rhs = wv cols (moving)
                    mm3(ps, xv_t, wv_t, slice(ti * P, (ti + 1) * P),
                        slice(dch * 512, (dch + 1) * 512))
                    # v' = 64*v: ps is x64 (weights) and bv arrives x64; the
                    # softmax ones-column is 64 so the scale cancels.
                    nc.vector.tensor_tensor(
                        v_sb[t][:, dch * 8:(dch + 1) * 8, 0:DK],
                        ps[:].rearrange("p (h d) -> p h d", h=8),
                        bv_b[:, dch * 512:(dch + 1) * 512].rearrange(
                            "p (h d) -> p h d", h=8),
                        OP.add)

    def load_qk(p1w, p2w, load_consts):
        """Staged Q/K loads keyed by p3 token-group, riding behind p3's own
        xv/wv traffic.  Order matches first use: stage0 feeds the post-p3
        Q-projection's hi terms (and the consts), stage1 the attention
        prologue's (wk_hi, xk_hi), then the lo parts."""
        xq_t = [p1w.tile([P, DCH, T], FP8, tag=f"xq{i}", name=f"xq{i}")
                for i in range(2)]
        wq_t = [p1w.tile([P, DCH, D], FP8, tag=f"wq{i}", name=f"wq{i}")
                for i in range(2)]
        xk_t = [p2w.tile([P, DCH, S], FP8, tag=f"xk{i}", name=f"xk{i}")
                for i in range(2)]
        wk_t = [p2w.tile([P, DCH, D], FP8, tag=f"wk{i}", name=f"wk{i}")
                for i in range(2)]
        stages = {
            0: lambda: (load_consts(),
                        nc.sync.dma_start(xq_t[0], chunked(xq2[0])),
                        nc.sync.dma_start(wq_t[0], chunked(wq2[0]))),
            1: lambda: (nc.sync.dma_start(xk_t[0], chunked(xk2[0])),
                        nc.sync.dma_start(wk_t[0], chunked(wk2[0]))),
            2: lambda: (nc.sync.dma_start(wq_t[1], chunked(wq2[1])),
                        nc.sync.dma_start(wk_t[1], chunked(wk2[1]))),
            3: lambda: (nc.sync.dma_start(xq_t[1], chunked(xq2[1])),
                        nc.sync.dma_start(xk_t[1], chunked(xk2[1]))),
        }
        return xq_t, wq_t, xk_t, wk_t, stages

    def emit_p2_attn(qT, v_sb, attn, bq_col, bk_col, xq_t, wq_t, xk_t, wk_t,
                     aK, aE, aR, acc, psS, psA):
        """Software-pipelined attention: 128 exp-units (head, score-group),
        ACT-paced via psS double-buffering; attnV(h) trails its last exp by
        2 units; the next pair's fp8 K- AND Q-projection chunks fill the
"""
        kp_tiles = {}
        exps = {}
        acc_open = {}
        TERMS = ((wk_t[0], xk_t[0]), (wk_t[1], xk_t[0]), (wk_t[0], xk_t[1]))
        QTERMS = ((wq_t[0], xq_t[0]), (wq_t[1], xq_t[0]), (wq_t[0], xq_t[1]))

        def emit_qproj_sub(m, term):
            """One term of deferred Q-projection for head-pair m."""
            if term == 0:
                acc_open["q", m] = acc.tile([P, 512], F32, tag="acc",
                                            name="acc")
            ps = acc_open["q", m]
            wb, xb = QTERMS[term]
            for j in range(NDR):
                mm(ps, wb[:, 2 * j:2 * j + 2, m * P:(m + 1) * P],
                   xb[:, 2 * j:2 * j + 2, :],
                   start=(term == 0 and j == 0),
                   stop=(term == 2 and j == NDR - 1), perf_mode=DR)
            if term == 2:
                nc.vector.tensor_scalar(qT[m], ps, bq_col[:, m:m + 1],
                                        1.0 / W8SCALE, OP.add, OP.mult)

        def emit_kproj_sub(p, nch, term):
            """One term (4 DR matmuls, ~0.43us PE) of K-projection chunk
            (p, nch) -- fine-grained so it can pack into the exp-paced
            stalls of the scores stream."""
            if nch == 0 and term == 0:
                kp_tiles[p] = aK.tile([P, S], F32R, tag="kp", name="kp")
            if term == 0:
                acc_open[p] = acc.tile([P, 512], F32, tag="acc", name="acc")
            ps = acc_open[p]
            wb, xb = TERMS[term]
            for j in range(NDR):
                mm(ps, wb[:, 2 * j:2 * j + 2, p * P:(p + 1) * P],
                   xb[:, 2 * j:2 * j + 2, nch * 512:(nch + 1) * 512],
                   start=(term == 0 and j == 0),
                   stop=(term == 2 and j == NDR - 1), perf_mode=DR)
            if term == 2:
                # kp = ps/64 + bk  (bk arrives x64)
                nc.vector.tensor_scalar(
                    kp_tiles[p][:, nch * 512:(nch + 1) * 512], ps,
                    bk_col[:, p:p + 1], 1.0 / W8SCALE, OP.add, OP.mult)

        def emit_unit(u):
            head, g = u // 8, u % 8
            p, hp = head // 2, head % 2
            lo, hi = hp * DK, (hp + 1) * DK
            kp = kp_tiles[p]
            ps = psS.tile([P, 1024], F32, tag="psS", name="psS")
            for j in range(2):
                m = 2 * g + j
                mm(ps[:, j * 512:(j + 1) * 512],
                   kp[lo:hi, m * P:(m + 1) * P],
                   qT[p][lo:hi, :], start=True, stop=True)
            e = aE.tile([P, 1024], BF16, tag="exp", name="exp")
            nc.scalar.activation(e, ps, AF.Exp)
            exps.setdefault(head, []).append(e)

        def emit_attnv(head):
            ex = exps.pop(head)
            pa = psA.tile([P, QS, DK + 1], F32, tag="pa", name="pa")
            # m outer so early exp tiles are released after 8 matmuls and
            # ACT can start on the next head's scores immediately
            loop = ([(m, q) for m in range(KC) for q in range(QS)]
                    if M_OUTER else
                    [(m, q) for q in range(QS) for m in range(KC)])
            for m, q in loop:
                mm(pa[:, q, :],
                   ex[m // 2][:, (m % 2) * 512 + q * P:
                              (m % 2) * 512 + (q + 1) * P],
                   v_sb[m][:, head, :],
                   start=(m == 0), stop=(m == KC - 1))
            rc = aR.tile([P, QS], F32, tag="rc", name="rc")
            nc.vector.reciprocal(rc, pa[:, :, DK:DK + 1])
            for q in range(QS):
                nc.vector.tensor_scalar_mul(
                    attn[q][:, head * DK:(head + 1) * DK],
                    pa[:, q, 0:DK], rc[:, q:q + 1])

        for nch in range(4):
            for term in range(3):
                emit_kproj_sub(0, nch, term)
        subs = []
        for w in range(1, H // 2):
            if w >= 3:
                subs += [("q", w, term) for term in range(3)]
            subs += [("k", w, nch, term) for nch in range(4)
                     for term in range(3)]

        def emit_sub(s):
            if s[0] == "q":
                emit_qproj_sub(s[1], s[2])
            else:
                emit_kproj_sub(s[1], s[2], s[3])

        sc = 0
        for idx in range(131):
            if idx < 128:
                emit_unit(idx)
            if idx % 8 != 1 and sc < len(subs) \
                    and subs[sc][1] <= idx // 16 + 1:
                emit_sub(subs[sc])
                sc += 1
            if idx >= 10 and (idx - 10) % 8 == 0:
                emit_attnv((idx - 10) // 8)
        while sc < len(subs):
            emit_sub(subs[sc])
            sc += 1
    def emit_ln_tr(attn, ffi, ffiT, eps_t, lng_b, lnb_b, ident_bf, lnp, psTr):
        for q in range(QS):
            stats = lnp.tile([P, 2, 6], F32, tag="stats", name="stats")
            for sg in range(2):
                nc.vector.bn_stats(stats[:, sg, :],
                                   attn[q][:, sg * 512:(sg + 1) * 512])
            mv = lnp.tile([P, 2], F32, tag="mv", name="mv")
            nc.vector.bn_aggr(mv, stats)
            std = lnp.tile([P, 1], F32, tag="std", name="std")
            nc.scalar.activation(std, mv[:, 1:2], AF.Sqrt, bias=eps_t)
            rstd = lnp.tile([P, 1], F32, tag="rstd", name="rstd")
            nc.vector.reciprocal(rstd, std)
            nc.vector.tensor_scalar(ffi[q], attn[q], mv[:, 0:1], rstd,
                                    OP.subtract, OP.mult)
            if ln_affine:
                nc.vector.tensor_mul(ffi[q], ffi[q], lng_b)
                nc.vector.tensor_add(ffi[q], ffi[q], lnb_b)
            for k in range(DCH):
                pt = psTr.tile([P, P], BF16 if BF16_TAIL else F32,
                               tag="ptr", name="ptr")
                nc.tensor.transpose(pt, ffi[q][:, k * P:(k + 1) * P],
                                    ident_bf[:, 0:P])
                nc.vector.tensor_copy(ffiT[k][:, q * P:(q + 1) * P], pt)

    def emit_ffn(ffi, ffiT, out_sb, b1_col, b2_b,
                 hp_, fw1, fw2, psH, psF, out_dma=None):
        """FFN1 in bf16 (W1 host-scaled 1/8 so hT = h/8 sits in fp8 range);
        hT is split hi/lo into fp8 (DVE relu -> ACT copy -> DVE residual) and
        FFN2 runs as 3-term fp8 DoubleRow against host-split 8*W2, so the
        product is exactly ff with no descale pass."""
        hT8 = [hp_.tile([P, FFC, T], FP8, tag=f"hT8{i}", name=f"hT8{i}")
               for i in range(2)]
        w1_sb = []
        for k in range(DCH):
            w1t = fw1.tile([P, FF], BF16, tag=f"w1_{k}", name=f"w1_{k}")
            nc.sync.dma_start(w1t, w1[k * P:(k + 1) * P, :])
            w1_sb.append(w1t)

        def w2_tile(i, g, half):
            t_ = fw2.tile([P, 2, 512], FP8, tag=f"w2{half}{i}",
                          name=f"w2{half}{i}")
            nc.sync.dma_start(
                t_, w22[i][2 * g * P:(2 * g + 2) * P,
                           half * 512:(half + 1) * 512].rearrange(
                    "(c p) n -> p c n", p=P))
            return t_

        def ffn2_pair(pss, g, w2h_t, w2l_t, on_q_done=None):
            terms = ((0, w2h_t), (1, w2h_t), (0, w2l_t))
            for q in range(QS):
                for tr, (hx, wx) in enumerate(terms):
                    mm(pss[q], hT8[hx][:, 2 * g:2 * g + 2, q * P:(q + 1) * P],
                       wx, start=(g == 0 and tr == 0),
                       stop=(g == FFC // 2 - 1 and tr == 2), perf_mode=DR)
                if on_q_done is not None:
                    on_q_done(q)

        pss0 = [psF.tile([P, 512], F32, tag="psF", name="psF")
                for _ in range(QS)]
        for fk in range(FFC):
            ps = psH.tile([P, T], F32, tag="psH", name="psH")
            for k in range(DCH):
                mm(ps, w1_sb[k][:, fk * P:(fk + 1) * P], ffiT[k],
                   start=(k == 0), stop=(k == DCH - 1))
            t_re = fw2.tile([P, T], F32, tag="tre", name="tre")
            nc.vector.tensor_scalar(t_re, ps, b1_col[:, fk:fk + 1], 0.0,
                                    OP.add, OP.max)
            nc.scalar.activation(hT8[0][:, fk, :], t_re, AF.Copy)
            nc.vector.tensor_tensor(hT8[1][:, fk, :], t_re,
                                    hT8[0][:, fk, :], OP.subtract)
            if fk % 2 == 1:
                g = fk // 2
                ffn2_pair(pss0, g, w2_tile(0, g, 0), w2_tile(1, g, 0))
        for q in range(QS):
            nc.vector.tensor_add(out_sb[q][:, 0:512], pss0[q],
                                 ffi[q][:, 0:512])
            if not b2_zero:
                nc.vector.tensor_add(out_sb[q][:, 0:512],
                                     out_sb[q][:, 0:512], b2_b[:, 0:512])
            if out_dma is not None:
                out_dma(q, 0)
        pss1 = [psF.tile([P, 512], F32, tag="psF", name="psF")
                for _ in range(QS)]

        def final_add(q):
            nc.vector.tensor_add(out_sb[q][:, 512:1024], pss1[q],
                                 ffi[q][:, 512:1024])
            if not b2_zero:
                nc.vector.tensor_add(out_sb[q][:, 512:1024],
                                     out_sb[q][:, 512:1024],
                                     b2_b[:, 512:1024])
            if out_dma is not None:
                out_dma(q, 1)

        for g in range(FFC // 2):
            ffn2_pair(pss1, g, w2_tile(0, g, 1), w2_tile(1, g, 1),
                      on_q_done=final_add if g == FFC // 2 - 1 else None)

    with tile.TileContext(nc) as tc:
        with (
            tc.tile_pool(name="const", bufs=1) as cp,
            tc.tile_pool(name="qTp", bufs=1) as qp,
            tc.tile_pool(name="attnp", bufs=1) as ap_,
            tc.tile_pool(name="p1w", bufs=1) as p1w,
            tc.tile_pool(name="accp", bufs=2, space="PSUM") as acc,
        ):
            ident = cp.tile([P, P], F32, tag="ident", name="ident")
            make_identity(nc, ident)
            ident_bf = cp.tile([P, P], BF16, tag="identb", name="identb")
            nc.vector.tensor_copy(ident_bf, ident)
            eps_t = cp.tile([P, 1], F32, tag="eps", name="eps")
            nc.vector.memset(eps_t, 1e-5)
            ones_t = cp.tile([P, H, 1], F32, tag="ones", name="ones")
            nc.vector.memset(ones_t, W8SCALE)
            bq_col = cp.tile([P, DCH], F32, tag="bqc", name="bqc")
            bk_col = cp.tile([P, DCH], F32, tag="bkc", name="bkc")
            b1_col = cp.tile([P, FFC], F32, tag="b1c", name="b1c")
            lng_b = cp.tile([P, D], F32, tag="lng", name="lng")
            lnb_b = cp.tile([P, D], F32, tag="lnb", name="lnb")
            bv_b = cp.tile([P, D], F32, tag="bvb", name="bvb")
            b2_b = cp.tile([P, D], F32, tag="b2b", name="b2b")

            def load_consts():
                # issued AFTER p1's operand DMAs: the bcasts are 4x512KB and
                # would otherwise interleave on the DMA engines ahead of
                # wq_hi, delaying the first matmul.  bv leads the gpsimd
                # queue (cumulative sem; its consumer is p3's first bias).
                nc.sync.dma_start(bq_col, bq[:].rearrange("(o p) -> p o", p=P))
                nc.sync.dma_start(bk_col, bk[:].rearrange("(o p) -> p o", p=P))
                nc.sync.dma_start(b1_col, b1[:].rearrange("(o p) -> p o", p=P))
                nc.gpsimd.dma_start(bv_b, _bcast_ap(bv[:]))
                nc.gpsimd.dma_start(lng_b, _bcast_ap(ln_g[:]))
                nc.gpsimd.dma_start(lnb_b, _bcast_ap(ln_b[:]))
                nc.gpsimd.dma_start(b2_b, _bcast_ap(b2[:]))

            qT = [qp.tile([P, T], F32R, tag=f"qT{m}", name=f"qT{m}")
                  for m in range(DCH)]
            attn = [ap_.tile([P, D], F32, tag=f"attn{q}", name=f"attn{q}")
                    for q in range(QS)]

            with tc.tile_pool(name="vp", bufs=1) as vp:
                v_sb = [vp.tile([P, H, DK + 1], BF16, tag=f"v{t}", name=f"v{t}")
                        for t in range(KC)]
                with tc.tile_pool(name="p2w", bufs=1) as p2w:
                    if "pa" in phases or "p1" in phases:
                        xq_t, wq_t, xk_t, wk_t, stages = load_qk(
                            p1w, p2w, load_consts)
                    else:
                        load_consts()
                    if "p3" in phases:
                        with (
                            tc.tile_pool(name="p3w", bufs=1) as p3w,
                            tc.tile_pool(name="p3x", bufs=3) as p3x,
                        ):
                            emit_p3(v_sb, bv_b, ones_t, p3w, p3x, acc,
                                    prefetch=stages)
                    else:
                        for tg in (0, 1, 2, 3):
                            stages[tg]()
                    if "p1" in phases:
                        emit_p1(qT, bq_col, xq_t, wq_t, acc)
                    if "pa" in phases:
                        with (
                            tc.tile_pool(name="aK", bufs=3) as aK,
                            tc.tile_pool(name="aE", bufs=15) as aE,
                            tc.tile_pool(name="aR", bufs=2) as aR,
                            tc.tile_pool(name="psS", bufs=2, space="PSUM") as psS,
                            tc.tile_pool(name="psA", bufs=2, space="PSUM") as psA,
                        ):
                            emit_p2_attn(qT, v_sb, attn, bq_col, bk_col,
                                         xq_t, wq_t, xk_t, wk_t,
                                         aK, aE, aR, acc, psS, psA)
                            # prewarm the Sqrt ACT table set so the switch
                            # isn't on the LayerNorm critical path
                            warm = aR.tile([P, 1], F32, tag="warm",
                                           name="warm")
                            nc.scalar.activation(warm, eps_t, AF.Sqrt)

            with (
                tc.tile_pool(name="ffip", bufs=1) as fip,
                tc.tile_pool(name="ffiTp", bufs=1) as ftp,
                tc.tile_pool(name="outp", bufs=1) as op_,
            ):
                ffi = [fip.tile([P, D], BF16 if BF16_TAIL else F32,
                                tag=f"ffi{q}", name=f"ffi{q}")
                       for q in range(QS)]
                ffiT = [ftp.tile([P, T], BF16, tag=f"ffiT{k}", name=f"ffiT{k}")
                        for k in range(DCH)]
                out_sb = [op_.tile([P, D], BF16 if BF16_TAIL else F32,
                                   tag=f"out{q}", name=f"out{q}")
                          for q in range(QS)]

                if "ln" in phases and "tr" in phases:
                    with (
                        tc.tile_pool(name="lnp", bufs=4) as lnp,
                        tc.tile_pool(name="psTr", bufs=4, space="PSUM") as psTr,
                    ):
                        emit_ln_tr(attn, ffi, ffiT, eps_t, lng_b, lnb_b,
                                   ident_bf if BF16_TAIL else ident,
                                   lnp, psTr)

                if "ffn" in phases:
                    with (
                        tc.tile_pool(name="hTp", bufs=1) as hp_,
                        tc.tile_pool(name="fw1", bufs=1) as fw1,
                        tc.tile_pool(name="fw2", bufs=4) as fw2,
                        tc.tile_pool(name="psH", bufs=2, space="PSUM") as psH,
                        tc.tile_pool(name="psF", bufs=4, space="PSUM") as psF,
                    ):
                        def out_dma(q, half):
                            sl = slice(half * 512, (half + 1) * 512)
                            nc.sync.dma_start(out[q * P:(q + 1) * P, sl],
                                              out_sb[q][:, sl])
                        emit_ffn(ffi, ffiT, out_sb, b1_col, b2_b,
                                 hp_, fw1, fw2, psH, psF, out_dma=out_dma)

    nc.compile()
    return nc


def _split8(a):
    """f32 array -> (hi, lo) float8_e4m3 pair with hi + lo ~= a."""
    import ml_dtypes
    hi = a.astype(ml_dtypes.float8_e4m3)
    lo = (a - hi.astype(np.float32)).astype(ml_dtypes.float8_e4m3)
    return hi, lo


def kernel(**inputs) -> np.ndarray:
    import ml_dtypes
    f32 = lambda a: np.asarray(a, dtype=np.float32)
    query, key, value = f32(inputs["query"]), f32(inputs["key"]), f32(inputs["value"])
    scale = 1.0 / np.sqrt(np.float32(DK))
    wqh, wql = _split8(np.ascontiguousarray(f32(inputs["Wq"]) * (scale * W8SCALE)))
    wkh, wkl = _split8(f32(inputs["Wk"]) * W8SCALE)
    wvh, wvl = _split8(f32(inputs["Wv"]) * W8SCALE)
    bq = f32(inputs["bq"]) * scale * W8SCALE
    bk = f32(inputs["bk"]) * W8SCALE
    bv = f32(inputs["bv"]) * W8SCALE
    # W1 scaled 1/8 so hT = h/8 lands in fp8 range; W2 x8 compensates exactly
    w1 = (f32(inputs["W1"]) * 0.125).astype(ml_dtypes.bfloat16)
    b1 = f32(inputs["b1"]) * 0.125
    w2h, w2l = _split8(f32(inputs["W2"]) * 8.0)
    b2 = f32(inputs["b2"])
    ln_g, ln_b = f32(inputs["ln_g"]), f32(inputs["ln_b"])

    ln_affine = not (np.all(ln_g == 1.0) and np.all(ln_b == 0.0))
    nc = build_program(ln_affine=ln_affine, b2_zero=not b2.any())

    shared = dict(wqh=wqh, wql=wql, wkh=wkh, wkl=wkl, wvh=wvh, wvl=wvl,
                  w1=w1, w2h=w2h, w2l=w2l, bq=bq, bk=bk, bv=bv,
                  b1=b1, b2=b2, ln_g=ln_g, ln_b=ln_b)
    xk_splits, xv_splits = {}, {}
    for b in range(B):
        xk_splits[b] = _split8(np.ascontiguousarray(key[b].T))
        xv_splits[b] = _split8(np.ascontiguousarray(value[b].T))
    in_maps = []
    for c in range(N_CORES):
        b = c // 4
        t0 = (c % 4) * T
        xqh, xql = _split8(np.ascontiguousarray(query[b, t0:t0 + T, :].T))
        in_maps.append(dict(
            xqh=xqh, xql=xql,
            xkh=xk_splits[b][0], xkl=xk_splits[b][1],
            xvh=xv_splits[b][0], xvl=xv_splits[b][1],
            **shared,
        ))

    res = run_bass_kernel_spmd(nc, in_maps, list(range(N_CORES)))
    out = np.empty((B, S, D), dtype=np.float32)
    for c in range(N_CORES):
        b = c // 4
        t0 = (c % 4) * T
        out[b, t0:t0 + T, :] = res.results[c]["out"].astype(np.float32)
    return out
